# revision 1
# baseline (speedup 1.0000x reference)
"""Trainium2 Bass kernel for nn_Cross_MultiAttention (8-head cross attention).

Sharding: one attention head per NeuronCore (8 heads / 8 cores).

Host folds the shared 1x1 input conv into each head's q/k/v projections
(Aq = wq_h @ w_in etc.), so each core:
  - projects q/k/v for its head directly from (x+pos) / (context+pos),
  - computes the full 5000x5000 attention for its head with scores kept
    TRANSPOSED (keys on partitions, queries on the free dim). Softmax is
    max-free (|scores/16| < ~4) and the denominator comes from an appended
    ones-column in V, so no cross-partition reductions are needed.
  - The probability tiles for a whole 1024-query stripe are kept in SBUF,
    and the P@V pass for stripe w-1 is interleaved with the QK^T pass for
    stripe w, so the tensor engine never waits on softmax.
  - applies its head's slice of the output projection to the UNNORMALIZED
    attention output and exports the per-query softmax denominator row.
Host divides each partial [256, 5000] by its denominator, sums the 8
partials, adds b_out, reshapes to [256, 50, 100].

dtypes: fp32 in DRAM except the mask (fp16 0/1); on-chip the big matmuls
run in float32r (full-rate PE mode), probabilities/V in fp16 (bounded).
"""

import numpy as np

import concourse.bacc as bacc
import concourse.tile as tile
import concourse.mybir as mybir
from concourse.bass_utils import run_bass_kernel_spmd

F32 = mybir.dt.float32
F32R = mybir.dt.float32r  # fp32 bits, full-rate PE streaming mode (rounded)
F16 = mybir.dt.float16
F8 = mybir.dt.float8e4
AF = mybir.ActivationFunctionType

EMB = 256
HEADS = 8
DEPTH = 32
IN_CH = 256
H, W = 50, 100
N_TOK = H * W  # 5000
SCALE = EMB ** (-0.5)  # 1/16


def _tiles(total, size):
    out = []
    p = 0
    while p < total:
        out.append((p, min(size, total - p)))
        p += size
    return out


def build_nc(n_tok=N_TOK, num_devices=8, wsz=1024, jsz=128):
    """Build the Bass module (same SPMD program for every core)."""
    nc = bacc.Bacc("TRN2", target_bir_lowering=False, debug=False,
                   num_devices=num_devices)

    D = DEPTH
    xp_d = nc.dram_tensor("xp", (IN_CH, n_tok), F32R, kind="ExternalInput").ap()
    cp_d = nc.dram_tensor("cp", (IN_CH, n_tok), F32R, kind="ExternalInput").ap()
    nmT_d = nc.dram_tensor("nmT", (n_tok, n_tok), F8, kind="ExternalInput").ap()
    AqT_d = nc.dram_tensor("AqT", (IN_CH, 4 * D), F32R, kind="ExternalInput").ap()
    cq_d = nc.dram_tensor("cq", (4 * D, 1), F32, kind="ExternalInput").ap()
    AkT_d = nc.dram_tensor("AkT", (IN_CH, 4 * D), F32R, kind="ExternalInput").ap()
    ck_d = nc.dram_tensor("ck", (4 * D, 1), F32, kind="ExternalInput").ap()
    AvT_d = nc.dram_tensor("AvT", (IN_CH, D), F32, kind="ExternalInput").ap()
    cvb_d = nc.dram_tensor("cvb", (128, D), F32, kind="ExternalInput").ap()
    woT_d = nc.dram_tensor("woT", (D, EMB), F32R, kind="ExternalInput").ap()
    y_d = nc.dram_tensor("y", (EMB, n_tok), F32, kind="ExternalOutput").ap()
    dn_d = nc.dram_tensor("dn", (1, n_tok), F32, kind="ExternalOutput").ap()

    ntiles = _tiles(n_tok, 512)   # 512-wide tiles (projections)
    wtiles = _tiles(n_tok, wsz)   # wide query stripes for the attention loop
    jtiles = _tiles(n_tok, jsz)   # key tiles (partition dim of scores)
    NJ = len(jtiles)
    NW = len(wtiles)

    with tile.TileContext(nc) as tc:
        with (
            tc.tile_pool(name="persist", bufs=1) as persist,
            tc.tile_pool(name="consts", bufs=1) as consts,
        ):
            # ---- constants to SBUF ----
            AqT_sb = consts.tile([128, 2, 4 * D], F32R)
            AkT_sb = consts.tile([128, 2, 4 * D], F32R)
            AvT_sb = consts.tile([128, 2, D], F32)
            for ct in range(2):
                nc.sync.dma_start(AqT_sb[:, ct, :], AqT_d[ct * 128:(ct + 1) * 128, :])
                nc.sync.dma_start(AkT_sb[:, ct, :], AkT_d[ct * 128:(ct + 1) * 128, :])
                nc.sync.dma_start(AvT_sb[:, ct, :], AvT_d[ct * 128:(ct + 1) * 128, :])
            cq_sb = consts.tile([4 * D, 1], F32)
            nc.sync.dma_start(cq_sb[:, :], cq_d[:, :])
            ck_sb = consts.tile([4 * D, 1], F32)
            nc.sync.dma_start(ck_sb[:, :], ck_d[:, :])
            cvb_sb = consts.tile([128, D], F32)
            nc.sync.dma_start(cvb_sb[:, :], cvb_d[:, :])
            woT_sb = consts.tile([D, EMB], F32R)
            nc.sync.dma_start(woT_sb[:, :], woT_d[:, :])

            # ---- persistent activations ----
            qT = persist.tile([4 * D, n_tok], F16)
            kT = persist.tile([4 * D, n_tok], F16)
            v_sb = persist.tile([128, NJ, D + 1], F16)  # [j % 128, jt, d | ones]
            ones_stage = consts.tile([128, NJ], F32)
            nc.any.memset(ones_stage[:, :], 1.0)
            nc.vector.tensor_copy(v_sb[:, :, D], ones_stage[:, :])
            # probability stripe: all NJ key-tiles for one query stripe
            p_store = persist.tile([128, NJ, wsz], F16)

            # ---- stage 1: project q/k/v straight from (x|context)+pos ----
            with (
                tc.tile_pool(name="proj_in", bufs=3) as proj_in,
                tc.tile_pool(name="qk_ps", bufs=2, space="PSUM") as qk_ps,
                tc.tile_pool(name="v_ps", bufs=2, space="PSUM") as v_ps,
            ):
                for (n0, ns) in ntiles:
                    img_t = proj_in.tile([128, 2, 512], F32R, name="img_t")
                    for ct in range(2):
                        nc.sync.dma_start(
                            img_t[:, ct, :ns],
                            xp_d[ct * 128:(ct + 1) * 128, n0:n0 + ns])
                    qps = qk_ps.tile([4 * D, 512], F32, name="qps")
                    for ct in range(2):
                        nc.tensor.matmul(qps[:, :ns], AqT_sb[:, ct, :],
                                         img_t[:, ct, :ns],
                                         start=(ct == 0), stop=(ct == 1))
                    nc.vector.tensor_scalar_add(qT[:, n0:n0 + ns], qps[:, :ns],
                                                cq_sb[:, :])

                for (n0, ns) in ntiles:
                    img_t = proj_in.tile([128, 2, 512], F32R, name="img_t")
                    for ct in range(2):
                        nc.sync.dma_start(
                            img_t[:, ct, :ns],
                            cp_d[ct * 128:(ct + 1) * 128, n0:n0 + ns])
                    kps = qk_ps.tile([4 * D, 512], F32, name="qps")
                    for ct in range(2):
                        nc.tensor.matmul(kps[:, :ns], AkT_sb[:, ct, :],
                                         img_t[:, ct, :ns],
                                         start=(ct == 0), stop=(ct == 1))
                    nc.vector.tensor_scalar_add(kT[:, n0:n0 + ns], kps[:, :ns],
                                                ck_sb[:, :])
                    # v projection for the j-tiles inside this 512 stripe
                    for (jj0, jjs) in _tiles(ns, jsz):
                        jt = (n0 + jj0) // jsz
                        vps = v_ps.tile([128, D], F32, name="vps")
                        for ct in range(2):
                            nc.tensor.matmul(
                                vps[:jjs, :],
                                img_t[:, ct, jj0:jj0 + jjs].bitcast(F32),
                                AvT_sb[:, ct, :],
                                start=(ct == 0), stop=(ct == 1))
                        nc.vector.tensor_add(v_sb[:jjs, jt, 0:D], vps[:jjs, :],
                                             cvb_sb[:jjs, :])

            # ---- stage 2: pipelined attention + output projection ----
            with (
                tc.tile_pool(name="s_ps", bufs=2, space="PSUM") as s_ps_pool,
                tc.tile_pool(name="av_ps", bufs=2, space="PSUM") as av_ps_pool,
                tc.tile_pool(name="m_sb", bufs=8) as m_pool,
                tc.tile_pool(name="out_sb", bufs=2) as out_pool,
            ):
                def epilogue(av, i0p, iszp):
                    # unnormalized head output, denominator row, partial
                    # output projection for a finished stripe
                    unn = out_pool.tile([D + 1, wsz], F32R, name="unn")
                    nc.vector.tensor_copy(unn[:, :iszp], av[:, :iszp])
                    nc.sync.dma_start(dn_d[:, i0p:i0p + iszp],
                                      unn[D:D + 1, :iszp].bitcast(F32))
                    for c2 in range(2):
                        for (h0, hs) in _tiles(iszp, 512):
                            yps = s_ps_pool.tile([128, 512], F32, name="yps",
                                                 tag="s")
                            nc.tensor.matmul(
                                yps[:, :hs],
                                woT_sb[:, c2 * 128:(c2 + 1) * 128],
                                unn[0:D, h0:h0 + hs],
                                start=True, stop=True)
                            ysb = out_pool.tile([128, 512], F32, name="ysb")
                            nc.vector.tensor_copy(ysb[:, :hs], yps[:, :hs])
                            nc.sync.dma_start(
                                y_d[c2 * 128:(c2 + 1) * 128,
                                    i0p + h0:i0p + h0 + hs],
                                ysb[:, :hs])

                av = None
                pending = None  # (av, i0, isz) of the just-finished stripe
                for w in range(NW + 1):
                    if w >= 1:
                        i0p, iszp = wtiles[w - 1]
                        av = av_ps_pool.tile([D + 1, wsz], F32, name="av")
                    for jt, (j0, js) in enumerate(jtiles):
                        if w >= 1:
                            # P@V' for the PREVIOUS stripe (operands ready)
                            for (h0, hs) in _tiles(iszp, 512):
                                nc.tensor.matmul(
                                    av[:, h0:h0 + hs],
                                    v_sb[:js, jt, :],
                                    p_store[:js, jt, h0:h0 + hs],
                                    start=(jt == 0), stop=(jt == NJ - 1))
                        if w < NW:
                            i0, isz = wtiles[w]
                            s = s_ps_pool.tile([128, wsz], F32, name="s")
                            for (h0, hs) in _tiles(isz, 512):
                                nc.tensor.matmul(
                                    s[:js, h0:h0 + hs],
                                    kT[:, j0:j0 + js],
                                    qT[:, i0 + h0:i0 + h0 + hs],
                                    start=True, stop=True)
                            nc.scalar.activation(
                                p_store[:js, jt, :isz], s[:js, :isz],
                                AF.Exp, scale=float(SCALE) / 4.0)
                            m = m_pool.tile([128, wsz], F8, name="m")
                            nc.sync.dma_start(m[:js, :isz],
                                              nmT_d[j0:j0 + js, i0:i0 + isz])
                            nc.vector.tensor_mul(p_store[:js, jt, :isz],
                                                 p_store[:js, jt, :isz],
                                                 m[:js, :isz])
                        if jt == 4 and pending is not None:
                            epilogue(*pending)
                            pending = None
                    if w >= 1:
                        pending = (av, i0p, iszp)
                if pending is not None:
                    epilogue(*pending)

    nc.compile()
    return nc


def make_pos(row_embed, col_embed):
    """[EMB, H*W]; first half col embeds, second half row embeds."""
    d2 = row_embed.shape[1]
    pos = np.empty((EMB, H, W), np.float32)
    pos[:d2] = col_embed[:W].T[:, None, :]      # [d2, 1, W] -> broadcast H
    pos[d2:] = row_embed[:H].T[:, :, None]      # [d2, H, 1] -> broadcast W
    return pos.reshape(EMB, H * W)


def make_in_maps(x, context, pad_mask, row_embed, col_embed, w_in, b_in,
                 wq, bq, wk, bk, wv, bv, w_out, n_heads=HEADS):
    f8 = np.float64
    x = np.asarray(x, np.float32)
    context = np.asarray(context, np.float32)
    pad_mask = np.asarray(pad_mask)
    row_embed = np.asarray(row_embed, np.float32)
    col_embed = np.asarray(col_embed, np.float32)
    w_in = np.asarray(w_in, f8)
    b_in = np.asarray(b_in, f8)
    w_out = np.asarray(w_out, np.float32)
    wq, bq = np.asarray(wq, f8), np.asarray(bq, f8)
    wk, bk = np.asarray(wk, f8), np.asarray(bk, f8)
    wv, bv = np.asarray(wv, f8), np.asarray(bv, f8)

    pos = make_pos(row_embed, col_embed)
    xp = np.ascontiguousarray(x.reshape(EMB, N_TOK) + pos)
    cp = np.ascontiguousarray(context.reshape(EMB, N_TOK) + pos)
    import ml_dtypes
    nmT = np.ascontiguousarray((~pad_mask[0]).T).astype(ml_dtypes.float8_e4m3)

    shared = {"xp": xp, "cp": cp, "nmT": nmT}
    in_maps = []
    for h in range(n_heads):
        sl = slice(h * DEPTH, (h + 1) * DEPTH)
        Aq = wq[sl] @ w_in          # [D, IN_CH]
        cq = wq[sl] @ b_in + bq[sl]
        Ak = wk[sl] @ w_in
        ck = wk[sl] @ b_in + bk[sl]
        Av = wv[sl] @ w_in
        cv = wv[sl] @ b_in + bv[sl]
        f32c = lambda a: np.ascontiguousarray(a.astype(np.float32))
        in_maps.append(dict(
            shared,
            AqT=f32c(np.tile(Aq.T, (1, 4))),
            cq=f32c(np.tile(cq.reshape(DEPTH, 1), (4, 1))),
            AkT=f32c(np.tile(Ak.T, (1, 4))),
            ck=f32c(np.tile(ck.reshape(DEPTH, 1), (4, 1))),
            AvT=f32c(Av.T),
            cvb=f32c(np.broadcast_to(cv, (128, DEPTH))),
            woT=np.ascontiguousarray(w_out[:, sl].T),
        ))
    return in_maps


_CACHE = {}


def kernel(x, context, pad_mask, row_embed, col_embed, w_in, b_in,
           wq, bq, wk, bk, wv, bv, w_out, b_out):
    if "nc" not in _CACHE:
        _CACHE["nc"] = build_nc()
    nc = _CACHE["nc"]
    in_maps = make_in_maps(x, context, pad_mask, row_embed, col_embed,
                           w_in, b_in, wq, bq, wk, bk, wv, bv, w_out)
    res = run_bass_kernel_spmd(nc, in_maps, core_ids=list(range(HEADS)))
    y = np.zeros((EMB, N_TOK), np.float64)
    for c in range(HEADS):
        r = res.results[c]
        y += r["y"].astype(np.float64) / r["dn"].astype(np.float64)
    y = (y + np.asarray(b_out, np.float64)[:, None]).astype(np.float32)
    return y.reshape(EMB, H, W)



# revision 10
# speedup vs baseline: 1.1431x; 1.1431x over previous
"""Trainium2 Bass kernel for nn_Cross_MultiAttention (8-head cross attention).

Sharding: one attention head per NeuronCore (8 heads / 8 cores).

Host folds the shared 1x1 input conv into each head's q/k/v projections
(Aq = wq_h @ w_in etc.), so each core:
  - projects q/k/v for its head directly from (x+pos) / (context+pos), f16,
  - computes the full 5000x5000 attention for its head with scores kept
    TRANSPOSED (keys on partitions, queries on the free dim). Softmax is
    max-free (|scores/16| < ~4); the denominator comes from an appended
    ones-column in V.
  - P@V runs 2-wide column-tiled on the PE array (128x64 mode): two key
    tiles stream concurrently, each with a [V | ones] 33-col stationary,
    halving the P@V wall time vs. full-array matmuls.
  - keys are host-padded to 5120 (mask=0 on pad keys) so all 40 key tiles
    are full 128 partitions.
  - the per-stripe loop runs PV for stripe w-1, then QK^T + exp + mask for
    stripe w; the exp (ScalarE) is the kernel bottleneck and runs ~100%.
  - mask is f16 so the DVE mask-multiply hits the 2x 16-bit perf mode.
  - epilogue folds the cross-column-group reduction into the output
    projection stationary (rows 0-31 and 64-95 both carry w_out slice).
Host divides each partial [256, 5000] by its denominator row, sums the 8
partials, adds b_out, reshapes to [256, 50, 100].
"""

import numpy as np

import concourse.bacc as bacc
import concourse.tile as tile
import concourse.mybir as mybir
from concourse.bass_utils import run_bass_kernel_spmd

F32 = mybir.dt.float32
F32R = mybir.dt.float32r  # fp32 bits, full-rate PE streaming mode (rounded)
F16 = mybir.dt.float16
AF = mybir.ActivationFunctionType

EMB = 256
HEADS = 8
DEPTH = 32
IN_CH = 256
H, W = 50, 100
N_TOK = H * W          # 5000 queries
N_KEY = 5120           # keys padded to a multiple of 128
SCALE = EMB ** (-0.5)  # 1/16
WSZ = 1024             # query stripe width
NJ = N_KEY // 128      # 40 key tiles
D = DEPTH


def _tiles(total, size):
    out = []
    p = 0
    while p < total:
        out.append((p, min(size, total - p)))
        p += size
    return out


def build_nc(num_devices=8):
    nc = bacc.Bacc("TRN2", target_bir_lowering=False, debug=False,
                   num_devices=num_devices)

    xp_d = nc.dram_tensor("xp", (IN_CH, N_TOK), F16, kind="ExternalInput").ap()
    cp_d = nc.dram_tensor("cp", (IN_CH, N_KEY), F16, kind="ExternalInput").ap()
    nmT_d = nc.dram_tensor("nmT", (N_KEY, N_TOK), F16, kind="ExternalInput").ap()
    AqT_d = nc.dram_tensor("AqT", (IN_CH, 4 * D), F16, kind="ExternalInput").ap()
    cq_d = nc.dram_tensor("cq", (4 * D, 1), F32, kind="ExternalInput").ap()
    AkT_d = nc.dram_tensor("AkT", (IN_CH, 4 * D), F16, kind="ExternalInput").ap()
    ck_d = nc.dram_tensor("ck", (4 * D, 1), F32, kind="ExternalInput").ap()
    AvT_d = nc.dram_tensor("AvT", (IN_CH, D), F16, kind="ExternalInput").ap()
    cvb_d = nc.dram_tensor("cvb", (128, D), F32, kind="ExternalInput").ap()
    ly_d = nc.dram_tensor("ly", (128, EMB), F32R, kind="ExternalInput").ap()
    sdn_d = nc.dram_tensor("sdn", (128, 128), F32R, kind="ExternalInput").ap()
    y_d = nc.dram_tensor("y", (EMB, N_TOK), F32, kind="ExternalOutput").ap()
    dn_d = nc.dram_tensor("dn", (1, N_TOK), F32, kind="ExternalOutput").ap()

    xtiles = _tiles(N_TOK, WSZ)   # 5 tiles (last 904) - queries
    ctiles = _tiles(N_KEY, WSZ)   # 5 full tiles - keys
    wtiles = xtiles               # query stripes for the attention loop
    NW = len(wtiles)

    with tile.TileContext(nc) as tc:
        with (
            tc.tile_pool(name="persist", bufs=1) as persist,
            tc.tile_pool(name="consts", bufs=1) as consts,
            tc.tile_pool(name="s_ps", bufs=2, space="PSUM") as s_ps,
            tc.tile_pool(name="av_ps", bufs=2, space="PSUM") as av_ps,
            tc.tile_pool(name="proj_in", bufs=3) as proj_in,
            tc.tile_pool(name="m_sb", bufs=12) as m_pool,
            tc.tile_pool(name="out_sb", bufs=3) as out_pool,
        ):
            # ---- constants to SBUF ----
            AqT_sb = consts.tile([128, 2, 4 * D], F16)
            AkT_sb = consts.tile([128, 2, 4 * D], F16)
            AvT_sb = consts.tile([128, 2, D], F16)
            for ct in range(2):
                nc.sync.dma_start(AqT_sb[:, ct, :], AqT_d[ct * 128:(ct + 1) * 128, :])
                nc.sync.dma_start(AkT_sb[:, ct, :], AkT_d[ct * 128:(ct + 1) * 128, :])
                nc.sync.dma_start(AvT_sb[:, ct, :], AvT_d[ct * 128:(ct + 1) * 128, :])
            cq_sb = consts.tile([4 * D, 1], F32)
            nc.sync.dma_start(cq_sb[:, :], cq_d[:, :])
            ck_sb = consts.tile([4 * D, 1], F32)
            nc.sync.dma_start(ck_sb[:, :], ck_d[:, :])
            cvb_sb = consts.tile([128, D], F32)
            nc.sync.dma_start(cvb_sb[:, :], cvb_d[:, :])
            ly_sb = consts.tile([128, EMB], F32R)
            nc.sync.dma_start(ly_sb[:, :], ly_d[:, :])
            sdn_sb = consts.tile([128, 128], F32R)
            nc.sync.dma_start(sdn_sb[:, :], sdn_d[:, :])
            # warm up the exp table load early (hides ~2.7us)
            dumm = consts.tile([128, 16], F32)
            nc.any.memset(dumm[:, :], 0.0)
            dummo = consts.tile([128, 16], F32)
            nc.scalar.activation(dummo[:, :], dumm[:, :], AF.Exp, scale=1.0)

            # ---- persistent activations ----
            qT = persist.tile([4 * D, N_TOK], F16)
            kT = persist.tile([4 * D, N_KEY], F16)
            v_sb = persist.tile([128, NJ, D + 1], F16)  # [j % 128, jt, d | ones]
            ones_stage = consts.tile([128, NJ], F32)
            nc.any.memset(ones_stage[:, :], 1.0)
            nc.vector.tensor_copy(v_sb[:, :, D], ones_stage[:, :])
            # probability stripe: all NJ key-tiles for one query stripe
            p_store = persist.tile([128, NJ, WSZ], F16)

            # zero the av PSUM banks once so never-written rows stay finite
            for _ in range(2):
                za = av_ps.tile([128, 512], F32, name="za", tag="ava")
                zb = av_ps.tile([128, 512], F32, name="zb", tag="avb")
                nc.vector.memset(za[:, :], 0.0)
                nc.vector.memset(zb[:, :], 0.0)

            # ---- stage 1: project q/k/v straight from (x|context)+pos ----
            def proj_q(n0, ns):
                img_t = proj_in.tile([128, 2, WSZ], F16, name="img_t")
                for ct in range(2):
                    nc.sync.dma_start(img_t[:, ct, :ns],
                                      xp_d[ct * 128:(ct + 1) * 128, n0:n0 + ns])
                qps = s_ps.tile([128, WSZ], F32, name="qps", tag="s")
                for ct in range(2):
                    for (h0, hs) in _tiles(ns, 512):
                        nc.tensor.matmul(qps[:, h0:h0 + hs], AqT_sb[:, ct, :],
                                         img_t[:, ct, h0:h0 + hs],
                                         start=(ct == 0), stop=(ct == 1))
                nc.vector.tensor_scalar_add(qT[:, n0:n0 + ns], qps[:, :ns],
                                            cq_sb[:, :])

            def proj_kv(n0, ns):
                img_t = proj_in.tile([128, 2, WSZ], F16, name="img_t")
                for ct in range(2):
                    nc.sync.dma_start(img_t[:, ct, :ns],
                                      cp_d[ct * 128:(ct + 1) * 128, n0:n0 + ns])
                kps = s_ps.tile([128, WSZ], F32, name="kps", tag="s")
                for ct in range(2):
                    for (h0, hs) in _tiles(ns, 512):
                        nc.tensor.matmul(kps[:, h0:h0 + hs], AkT_sb[:, ct, :],
                                         img_t[:, ct, h0:h0 + hs],
                                         start=(ct == 0), stop=(ct == 1))
                nc.vector.tensor_scalar_add(kT[:, n0:n0 + ns], kps[:, :ns],
                                            ck_sb[:, :])
                # v projection for the j-tiles inside this stripe
                for (jj0, jjs) in _tiles(ns, 128):
                    jt = (n0 + jj0) // 128
                    vps = av_ps.tile([128, 512], F32, name="vps", tag="ava")
                    for ct in range(2):
                        nc.tensor.matmul(vps[:, 0:D],
                                         img_t[:, ct, jj0:jj0 + jjs],
                                         AvT_sb[:, ct, :],
                                         start=(ct == 0), stop=(ct == 1))
                    nc.vector.tensor_add(v_sb[:, jt, 0:D], vps[:, 0:D],
                                         cvb_sb[:, :])

            # stripe-0 queries first, then all keys, then remaining queries
            proj_q(*xtiles[0])
            for (n0, ns) in ctiles:
                proj_kv(n0, ns)
            for (n0, ns) in xtiles[1:]:
                proj_q(n0, ns)

            # ---- stage 2: pipelined attention + output projection ----
            for w in range(NW + 1):
                # PV phase for stripe w-1 (probabilities are ready)
                if w >= 1:
                    i0p, iszp = wtiles[w - 1]
                    for (h0, hs) in _tiles(iszp, 512):
                        av_a = av_ps.tile([128, 512], F32, name="av_a", tag="ava")
                        av_b = av_ps.tile([128, 512], F32, name="av_b", tag="avb")
                        np2 = NJ // 2
                        for jp in range(np2):
                            ja, jb = 2 * jp, 2 * jp + 1
                            nc.tensor.matmul(
                                av_a[0:D + 1, :hs], v_sb[:, ja, :],
                                p_store[:, ja, h0:h0 + hs],
                                start=(jp == 0), stop=(jp == np2 - 1),
                                tile_position=(0, 0))
                            nc.tensor.matmul(
                                av_b[64:64 + D + 1, :hs], v_sb[:, jb, :],
                                p_store[:, jb, h0:h0 + hs],
                                start=(jp == 0), stop=(jp == np2 - 1),
                                tile_position=(0, 64))
                        av2sb = out_pool.tile([128, 512], F32R, name="av2sb",
                                              tag="av2", bufs=2)
                        nc.vector.tensor_copy(av2sb[0:64, :hs], av_a[0:64, :hs])
                        nc.vector.tensor_copy(av2sb[64:128, :hs],
                                              av_b[64:128, :hs])
                        for c2 in range(2):
                            yps = s_ps.tile([128, WSZ], F32, name="yps", tag="s")
                            nc.tensor.matmul(
                                yps[:, :hs],
                                ly_sb[:, c2 * 128:(c2 + 1) * 128],
                                av2sb[:, :hs], start=True, stop=True)
                            ysb = out_pool.tile([128, 512], F32, name="ysb")
                            nc.vector.tensor_copy(ysb[:, :hs], yps[:, :hs])
                            nc.sync.dma_start(
                                y_d[c2 * 128:(c2 + 1) * 128,
                                    i0p + h0:i0p + h0 + hs],
                                ysb[:, :hs])
                        # denominator: rows 32 (even jt) + 96 (odd jt); the
                        # stationary's col 32 picks them, all other cols are
                        # zero so av_a's zero rows stay zero after this write
                        nc.tensor.matmul(av_a[:, :hs], sdn_sb[:, :],
                                         av2sb[:, :hs], start=True, stop=True)
                        dnsb = out_pool.tile([1, 512], F32, name="dnsb")
                        nc.vector.tensor_copy(dnsb[:, :hs], av_a[32:33, :hs])
                        nc.sync.dma_start(dn_d[:, i0p + h0:i0p + h0 + hs],
                                          dnsb[:, :hs])
                # QK phase for stripe w
                if w < NW:
                    i0, isz = wtiles[w]
                    for jt in range(NJ):
                        j0 = jt * 128
                        s = s_ps.tile([128, WSZ], F32, name="s", tag="s")
                        for (h0, hs) in _tiles(isz, 512):
                            nc.tensor.matmul(
                                s[:, h0:h0 + hs], kT[:, j0:j0 + 128],
                                qT[:, i0 + h0:i0 + h0 + hs],
                                start=True, stop=True)
                        nc.scalar.activation(p_store[:, jt, :isz], s[:, :isz],
                                             AF.Exp, scale=float(SCALE) / 4.0)
                        m = m_pool.tile([128, WSZ], F16, name="m")
                        nc.sync.dma_start(m[:, :isz],
                                          nmT_d[j0:j0 + 128, i0:i0 + isz])
                        nc.vector.tensor_mul(p_store[:, jt, :isz],
                                             p_store[:, jt, :isz],
                                             m[:, :isz])

    nc.compile()
    return nc


def make_pos(row_embed, col_embed):
    """[EMB, H*W]; first half col embeds, second half row embeds."""
    d2 = row_embed.shape[1]
    pos = np.empty((EMB, H, W), np.float32)
    pos[:d2] = col_embed[:W].T[:, None, :]      # [d2, 1, W] -> broadcast H
    pos[d2:] = row_embed[:H].T[:, :, None]      # [d2, H, 1] -> broadcast W
    return pos.reshape(EMB, H * W)


def make_in_maps(x, context, pad_mask, row_embed, col_embed, w_in, b_in,
                 wq, bq, wk, bk, wv, bv, w_out, n_heads=HEADS):
    f8 = np.float64
    x = np.asarray(x, np.float32)
    context = np.asarray(context, np.float32)
    pad_mask = np.asarray(pad_mask)
    row_embed = np.asarray(row_embed, np.float32)
    col_embed = np.asarray(col_embed, np.float32)
    w_in = np.asarray(w_in, f8)
    b_in = np.asarray(b_in, f8)
    w_out = np.asarray(w_out, np.float32)
    wq, bq = np.asarray(wq, f8), np.asarray(bq, f8)
    wk, bk = np.asarray(wk, f8), np.asarray(bk, f8)
    wv, bv = np.asarray(wv, f8), np.asarray(bv, f8)

    pos = make_pos(row_embed, col_embed)
    xp = (x.reshape(EMB, N_TOK) + pos).astype(np.float16)
    cp = np.zeros((EMB, N_KEY), np.float16)
    cp[:, :N_TOK] = (context.reshape(EMB, N_TOK) + pos).astype(np.float16)
    nmT = np.zeros((N_KEY, N_TOK), np.float16)
    nmT[:N_TOK, :] = (~pad_mask[0]).T.astype(np.float16)

    shared = {"xp": xp, "cp": cp, "nmT": nmT}
    in_maps = []
    for h in range(n_heads):
        sl = slice(h * DEPTH, (h + 1) * DEPTH)
        Aq = wq[sl] @ w_in          # [D, IN_CH]
        cq = wq[sl] @ b_in + bq[sl]
        Ak = wk[sl] @ w_in
        ck = wk[sl] @ b_in + bk[sl]
        Av = wv[sl] @ w_in
        cv = wv[sl] @ b_in + bv[sl]
        f16c = lambda a: np.ascontiguousarray(a.astype(np.float16))
        f32c = lambda a: np.ascontiguousarray(a.astype(np.float32))
        ly = np.zeros((128, EMB), np.float32)
        ly[0:D, :] = w_out[:, sl].T
        ly[64:64 + D, :] = w_out[:, sl].T
        sdn = np.zeros((128, 128), np.float32)
        sdn[D, D] = 1.0
        sdn[64 + D, D] = 1.0
        in_maps.append(dict(
            shared,
            AqT=f16c(np.tile(Aq.T, (1, 4))),
            cq=f32c(np.tile(cq.reshape(DEPTH, 1), (4, 1))),
            AkT=f16c(np.tile(Ak.T, (1, 4))),
            ck=f32c(np.tile(ck.reshape(DEPTH, 1), (4, 1))),
            AvT=f16c(Av.T),
            cvb=f32c(np.broadcast_to(cv, (128, DEPTH))),
            ly=ly,
            sdn=sdn,
        ))
    return in_maps


_CACHE = {}


def kernel(x, context, pad_mask, row_embed, col_embed, w_in, b_in,
           wq, bq, wk, bk, wv, bv, w_out, b_out):
    if "nc" not in _CACHE:
        _CACHE["nc"] = build_nc()
    nc = _CACHE["nc"]
    in_maps = make_in_maps(x, context, pad_mask, row_embed, col_embed,
                           w_in, b_in, wq, bq, wk, bk, wv, bv, w_out)
    res = run_bass_kernel_spmd(nc, in_maps, core_ids=list(range(HEADS)))
    y = np.zeros((EMB, N_TOK), np.float64)
    for c in range(HEADS):
        r = res.results[c]
        y += r["y"].astype(np.float64) / r["dn"].astype(np.float64)
    y = (y + np.asarray(b_out, np.float64)[:, None]).astype(np.float32)
    return y.reshape(EMB, H, W)


# revision 13
# speedup vs baseline: 1.2195x; 1.0668x over previous
"""Trainium2 Bass kernel for nn_Cross_MultiAttention (8-head cross attention).

Sharding: one attention head per NeuronCore (8 heads / 8 cores).

Host folds the shared 1x1 input conv into each head's q/k/v projections
(Aq = wq_h @ w_in etc.), so each core:
  - projects q/k/v for its head directly from (x+pos) / (context+pos), f16,
  - computes the full 5000x5000 attention for its head with scores kept
    TRANSPOSED (keys on partitions, queries on the free dim). Softmax is
    max-free (|scores/16| < ~4); the denominator comes from an appended
    ones-column in V.
  - P@V runs 2-wide column-tiled on the PE array (128x64 mode): two key
    tiles stream concurrently, each with a [V | ones] 33-col stationary.
    PV matmul pairs are interleaved into the QK loop of the next stripe so
    the ScalarE exp stream (the bottleneck) never starves.
  - keys are host-padded to 5120 (mask=0 on pad keys) so all 40 key tiles
    are full 128 partitions.
  - mask is f16 (DVE 2x 16-bit perf mode) and host-swizzled so each
    partition's bytes are DRAM-contiguous per stripe: DMA runs in 8-keytile
    chunks at ~16KB/descriptor instead of 2KB rows (the packet rate, not
    bandwidth, is the DMA ceiling).
  - epilogue folds the cross-column-group reduction into the output
    projection stationary (rows 0-31 and 64-95 both carry w_out slice).
Host divides each partial [256, 5000] by its denominator row, sums the 8
partials, adds b_out, reshapes to [256, 50, 100].
"""

import numpy as np

import concourse.bacc as bacc
import concourse.tile as tile
import concourse.mybir as mybir
from concourse.bass_utils import run_bass_kernel_spmd

F32 = mybir.dt.float32
F32R = mybir.dt.float32r  # fp32 bits, full-rate PE streaming mode (rounded)
F16 = mybir.dt.float16
AF = mybir.ActivationFunctionType

EMB = 256
HEADS = 8
DEPTH = 32
IN_CH = 256
H, W = 50, 100
N_TOK = H * W          # 5000 queries
N_KEY = 5120           # keys padded to a multiple of 128
NQP = 5120             # queries padded (mask layout only)
SCALE = EMB ** (-0.5)  # 1/16
WSZ = 1024             # query stripe width
NJ = N_KEY // 128      # 40 key tiles
NW = 5                 # query stripes
CH = 8                 # key tiles per mask DMA chunk
D = DEPTH


def _tiles(total, size):
    out = []
    p = 0
    while p < total:
        out.append((p, min(size, total - p)))
        p += size
    return out


def build_nc(num_devices=8):
    nc = bacc.Bacc("TRN2", target_bir_lowering=False, debug=False,
                   num_devices=num_devices)

    # packed inputs: per-partition bytes contiguous in DRAM
    xp_d = nc.dram_tensor("xp", (128, NW, 2, WSZ), F16, kind="ExternalInput").ap()
    cp_d = nc.dram_tensor("cp", (128, NW, 2, WSZ), F16, kind="ExternalInput").ap()
    nmT_d = nc.dram_tensor("nmT", (128, NW, NJ, WSZ), F16, kind="ExternalInput").ap()
    AqT_d = nc.dram_tensor("AqT", (IN_CH, 4 * D), F16, kind="ExternalInput").ap()
    cq_d = nc.dram_tensor("cq", (4 * D, 1), F32, kind="ExternalInput").ap()
    AkT_d = nc.dram_tensor("AkT", (IN_CH, 4 * D), F16, kind="ExternalInput").ap()
    ck_d = nc.dram_tensor("ck", (4 * D, 1), F32, kind="ExternalInput").ap()
    AvT_d = nc.dram_tensor("AvT", (IN_CH, D), F16, kind="ExternalInput").ap()
    cvb_d = nc.dram_tensor("cvb", (128, D), F32, kind="ExternalInput").ap()
    ly_d = nc.dram_tensor("ly", (128, EMB), F32R, kind="ExternalInput").ap()
    sdn_d = nc.dram_tensor("sdn", (128, 128), F32R, kind="ExternalInput").ap()
    y_d = nc.dram_tensor("y", (EMB, N_TOK), F32, kind="ExternalOutput").ap()
    dn_d = nc.dram_tensor("dn", (1, N_TOK), F32, kind="ExternalOutput").ap()

    wtiles = _tiles(N_TOK, WSZ)   # query stripes (last = 904)

    with tile.TileContext(nc) as tc:
        with (
            tc.tile_pool(name="persist", bufs=1) as persist,
            tc.tile_pool(name="consts", bufs=1) as consts,
            tc.tile_pool(name="s_ps", bufs=2, space="PSUM") as s_ps,
            tc.tile_pool(name="av_ps", bufs=2, space="PSUM") as av_ps,
            tc.tile_pool(name="proj_in", bufs=3) as proj_in,
            tc.tile_pool(name="m_sb", bufs=3) as m_pool,
            tc.tile_pool(name="out_sb", bufs=3) as out_pool,
        ):
            # ---- constants to SBUF ----
            AqT_sb = consts.tile([128, 2, 4 * D], F16)
            AkT_sb = consts.tile([128, 2, 4 * D], F16)
            AvT_sb = consts.tile([128, 2, D], F16)
            for ct in range(2):
                nc.sync.dma_start(AqT_sb[:, ct, :], AqT_d[ct * 128:(ct + 1) * 128, :])
                nc.sync.dma_start(AkT_sb[:, ct, :], AkT_d[ct * 128:(ct + 1) * 128, :])
                nc.sync.dma_start(AvT_sb[:, ct, :], AvT_d[ct * 128:(ct + 1) * 128, :])
            cq_sb = consts.tile([4 * D, 1], F32)
            nc.sync.dma_start(cq_sb[:, :], cq_d[:, :])
            ck_sb = consts.tile([4 * D, 1], F32)
            nc.sync.dma_start(ck_sb[:, :], ck_d[:, :])
            cvb_sb = consts.tile([128, D], F32)
            nc.sync.dma_start(cvb_sb[:, :], cvb_d[:, :])
            ly_sb = consts.tile([128, EMB], F32R)
            nc.sync.dma_start(ly_sb[:, :], ly_d[:, :])
            sdn_sb = consts.tile([128, 128], F32R)
            nc.sync.dma_start(sdn_sb[:, :], sdn_d[:, :])

            # warm up the exp table load early (hides ~2.7us)
            dumm = consts.tile([128, 16], F32)
            nc.any.memset(dumm[:, :], 0.0)
            dummo = consts.tile([128, 16], F32)
            nc.scalar.activation(dummo[:, :], dumm[:, :], AF.Exp, scale=1.0)

            # ---- persistent activations ----
            qT = persist.tile([4 * D, N_TOK], F16)
            kT = persist.tile([4 * D, N_KEY], F16)
            v_sb = persist.tile([128, NJ, D + 1], F16)  # [j % 128, jt, d | ones]
            ones_stage = consts.tile([128, NJ], F32)
            nc.any.memset(ones_stage[:, :], 1.0)
            nc.vector.tensor_copy(v_sb[:, :, D], ones_stage[:, :])
            # probability stripe: all NJ key-tiles for one query stripe
            p_store = persist.tile([128, NJ, WSZ], F16)

            # zero the av PSUM banks once so never-written rows stay finite
            for _ in range(2):
                za = av_ps.tile([128, 512], F32, name="za", tag="ava")
                zb = av_ps.tile([128, 512], F32, name="zb", tag="avb")
                nc.vector.memset(za[:, :], 0.0)
                nc.vector.memset(zb[:, :], 0.0)

            # ---- stage 1: project q/k/v straight from (x|context)+pos ----
            def proj_q(n):
                n0, ns = wtiles[n]
                img_t = proj_in.tile([128, 2, WSZ], F16, name="img_t")
                nc.sync.dma_start(img_t[:, :, :], xp_d[:, n, :, :])
                qps = s_ps.tile([128, WSZ], F32, name="qps", tag="s")
                for ct in range(2):
                    for (h0, hs) in _tiles(ns, 512):
                        nc.tensor.matmul(qps[:, h0:h0 + hs], AqT_sb[:, ct, :],
                                         img_t[:, ct, h0:h0 + hs],
                                         start=(ct == 0), stop=(ct == 1))
                nc.vector.tensor_scalar_add(qT[:, n0:n0 + ns], qps[:, :ns],
                                            cq_sb[:, :])

            def proj_kv(n):
                n0 = n * WSZ
                img_t = proj_in.tile([128, 2, WSZ], F16, name="img_t")
                nc.sync.dma_start(img_t[:, :, :], cp_d[:, n, :, :])
                kps = s_ps.tile([128, WSZ], F32, name="kps", tag="s")
                for ct in range(2):
                    for h0 in (0, 512):
                        nc.tensor.matmul(kps[:, h0:h0 + 512], AkT_sb[:, ct, :],
                                         img_t[:, ct, h0:h0 + 512],
                                         start=(ct == 0), stop=(ct == 1))
                nc.vector.tensor_scalar_add(kT[:, n0:n0 + WSZ], kps[:, :],
                                            ck_sb[:, :])
                # v projection for the j-tiles inside this stripe
                for jj0 in range(0, WSZ, 128):
                    jt = (n0 + jj0) // 128
                    vps = av_ps.tile([128, 512], F32, name="vps", tag="ava")
                    for ct in range(2):
                        nc.tensor.matmul(vps[:, 0:D],
                                         img_t[:, ct, jj0:jj0 + 128],
                                         AvT_sb[:, ct, :],
                                         start=(ct == 0), stop=(ct == 1))
                    nc.vector.tensor_add(v_sb[:, jt, 0:D], vps[:, 0:D],
                                         cvb_sb[:, :])

            # stripe-0 queries, then all keys; remaining queries are issued
            # inside the w=0 attention loop (spread over jt steps)
            proj_q(0)
            for n in range(NW):
                proj_kv(n)

            # ---- stage 2: pipelined attention + output projection ----
            for w in range(NW + 1):
                avab = None
                if w >= 1:
                    i0p, iszp = wtiles[w - 1]
                    phalf = _tiles(iszp, 512)
                    # one av pair per 512-half; both halves live concurrently
                    avab = [(av_ps.tile([128, 512], F32, name="av_a", tag="ava"),
                             av_ps.tile([128, 512], F32, name="av_b", tag="avb"))
                            for _ in phalf]
                i0, isz = wtiles[w] if w < NW else (0, 0)

                def load_chunk(c):
                    mchk = m_pool.tile([128, CH, WSZ], F16, name="mchk")
                    nc.sync.dma_start(mchk[:, :, :],
                                      nmT_d[:, w, c * CH:(c + 1) * CH, :])
                    return mchk

                cur_chk = nxt_chk = None
                for jt in range(NJ):
                    if w < NW:
                        # mask chunk prefetch, one chunk ahead
                        if jt == 0:
                            cur_chk = load_chunk(0)
                            nxt_chk = load_chunk(1)
                        elif jt % CH == 0:
                            cur_chk = nxt_chk
                            if (jt // CH + 1) * CH < NJ:
                                nxt_chk = load_chunk(jt // CH + 1)
                    # PV pair for the previous stripe, interleaved; must be
                    # issued BEFORE this jt's exp overwrites p_store[jt]
                    if w >= 1 and jt % 2 == 0:
                        jp = jt // 2
                        np2 = NJ // 2
                        for hi, (h0, hs) in enumerate(phalf):
                            av_a, av_b = avab[hi]
                            nc.tensor.matmul(
                                av_a[0:D + 1, :hs], v_sb[:, jt, :],
                                p_store[:, jt, h0:h0 + hs],
                                start=(jp == 0), stop=(jp == np2 - 1),
                                tile_position=(0, 0), skip_group_check=True)
                            nc.tensor.matmul(
                                av_b[64:64 + D + 1, :hs], v_sb[:, jt + 1, :],
                                p_store[:, jt + 1, h0:h0 + hs],
                                start=(jp == 0), stop=(jp == np2 - 1),
                                tile_position=(0, 64), skip_group_check=True)
                    if w < NW:
                        j0 = jt * 128
                        s = s_ps.tile([128, WSZ], F32, name="s", tag="s")
                        for (h0, hs) in _tiles(isz, 512):
                            nc.tensor.matmul(
                                s[:, h0:h0 + hs], kT[:, j0:j0 + 128],
                                qT[:, i0 + h0:i0 + h0 + hs],
                                start=True, stop=True)
                        nc.scalar.activation(p_store[:, jt, :isz], s[:, :isz],
                                             AF.Exp, scale=float(SCALE) / 4.0)
                        nc.vector.tensor_mul(p_store[:, jt, :isz],
                                             p_store[:, jt, :isz],
                                             cur_chk[:, jt % CH, :isz])
                    # remaining query projections, spread through stripe 0
                    if w == 0 and jt >= 10 and jt < 10 + 2 * (NW - 1) and jt % 2 == 0:
                        proj_q((jt - 10) // 2 + 1)
                # epilogue for stripe w-1
                if w >= 1:
                    for hi, (h0, hs) in enumerate(phalf):
                        av_a, av_b = avab[hi]
                        av2sb = out_pool.tile([128, 512], F32R, name="av2sb",
                                              tag="av2", bufs=2)
                        nc.vector.tensor_copy(av2sb[0:64, :hs], av_a[0:64, :hs])
                        nc.vector.tensor_copy(av2sb[64:128, :hs],
                                              av_b[64:128, :hs])
                        for c2 in range(2):
                            yps = s_ps.tile([128, WSZ], F32, name="yps", tag="s")
                            nc.tensor.matmul(
                                yps[:, :hs],
                                ly_sb[:, c2 * 128:(c2 + 1) * 128],
                                av2sb[:, :hs], start=True, stop=True)
                            ysb = out_pool.tile([128, 512], F32, name="ysb")
                            nc.vector.tensor_copy(ysb[:, :hs], yps[:, :hs])
                            nc.sync.dma_start(
                                y_d[c2 * 128:(c2 + 1) * 128,
                                    i0p + h0:i0p + h0 + hs],
                                ysb[:, :hs])
                        # denominator via stationary col 32 = rows 32 + 96
                        nc.tensor.matmul(av_a[:, :hs], sdn_sb[:, :],
                                         av2sb[:, :hs], start=True, stop=True)
                        dnsb = out_pool.tile([1, 512], F32, name="dnsb")
                        nc.vector.tensor_copy(dnsb[:, :hs], av_a[32:33, :hs])
                        nc.sync.dma_start(dn_d[:, i0p + h0:i0p + h0 + hs],
                                          dnsb[:, :hs])

    nc.compile()
    return nc


def make_pos(row_embed, col_embed):
    """[EMB, H*W]; first half col embeds, second half row embeds."""
    d2 = row_embed.shape[1]
    pos = np.empty((EMB, H, W), np.float32)
    pos[:d2] = col_embed[:W].T[:, None, :]      # [d2, 1, W] -> broadcast H
    pos[d2:] = row_embed[:H].T[:, :, None]      # [d2, H, 1] -> broadcast W
    return pos.reshape(EMB, H * W)


def _pack_img(a):
    # [256, ncol] f32 -> [128, ntile, 2, WSZ] f16, per-partition contiguous
    ncol = a.shape[1]
    nt = (ncol + WSZ - 1) // WSZ
    out = np.zeros((2, 128, nt, WSZ), np.float16)
    a16 = a.astype(np.float16)
    for ct in range(2):
        flat = np.zeros((128, nt * WSZ), np.float16)
        flat[:, :ncol] = a16[ct * 128:(ct + 1) * 128]
        out[ct] = flat.reshape(128, nt, WSZ)
    return np.ascontiguousarray(out.transpose(1, 2, 0, 3))


def make_in_maps(x, context, pad_mask, row_embed, col_embed, w_in, b_in,
                 wq, bq, wk, bk, wv, bv, w_out, n_heads=HEADS):
    f8 = np.float64
    x = np.asarray(x, np.float32)
    context = np.asarray(context, np.float32)
    pad_mask = np.asarray(pad_mask)
    row_embed = np.asarray(row_embed, np.float32)
    col_embed = np.asarray(col_embed, np.float32)
    w_in = np.asarray(w_in, f8)
    b_in = np.asarray(b_in, f8)
    w_out = np.asarray(w_out, np.float32)
    wq, bq = np.asarray(wq, f8), np.asarray(bq, f8)
    wk, bk = np.asarray(wk, f8), np.asarray(bk, f8)
    wv, bv = np.asarray(wv, f8), np.asarray(bv, f8)

    pos = make_pos(row_embed, col_embed)
    xp = _pack_img(x.reshape(EMB, N_TOK) + pos)
    cp = _pack_img(context.reshape(EMB, N_TOK) + pos)
    # mask: [5120 keys, 5120 queries] -> [128, NW, NJ, WSZ] f16 swizzle
    nm = np.zeros((N_KEY, NQP), np.float16)
    nm[:N_TOK, :N_TOK] = (~pad_mask[0]).T.astype(np.float16)
    nmT = np.ascontiguousarray(
        nm.reshape(NJ, 128, NW, WSZ).transpose(1, 2, 0, 3))

    shared = {"xp": xp, "cp": cp, "nmT": nmT}
    in_maps = []
    for h in range(n_heads):
        sl = slice(h * DEPTH, (h + 1) * DEPTH)
        Aq = wq[sl] @ w_in          # [D, IN_CH]
        cq = wq[sl] @ b_in + bq[sl]
        Ak = wk[sl] @ w_in
        ck = wk[sl] @ b_in + bk[sl]
        Av = wv[sl] @ w_in
        cv = wv[sl] @ b_in + bv[sl]
        f16c = lambda a: np.ascontiguousarray(a.astype(np.float16))
        f32c = lambda a: np.ascontiguousarray(a.astype(np.float32))
        ly = np.zeros((128, EMB), np.float32)
        ly[0:D, :] = w_out[:, sl].T
        ly[64:64 + D, :] = w_out[:, sl].T
        sdn = np.zeros((128, 128), np.float32)
        sdn[D, D] = 1.0
        sdn[64 + D, D] = 1.0
        in_maps.append(dict(
            shared,
            AqT=f16c(np.tile(Aq.T, (1, 4))),
            cq=f32c(np.tile(cq.reshape(DEPTH, 1), (4, 1))),
            AkT=f16c(np.tile(Ak.T, (1, 4))),
            ck=f32c(np.tile(ck.reshape(DEPTH, 1), (4, 1))),
            AvT=f16c(Av.T),
            cvb=f32c(np.broadcast_to(cv, (128, DEPTH))),
            ly=ly,
            sdn=sdn,
        ))
    return in_maps


_CACHE = {}


def kernel(x, context, pad_mask, row_embed, col_embed, w_in, b_in,
           wq, bq, wk, bk, wv, bv, w_out, b_out):
    if "nc" not in _CACHE:
        _CACHE["nc"] = build_nc()
    nc = _CACHE["nc"]
    in_maps = make_in_maps(x, context, pad_mask, row_embed, col_embed,
                           w_in, b_in, wq, bq, wk, bk, wv, bv, w_out)
    res = run_bass_kernel_spmd(nc, in_maps, core_ids=list(range(HEADS)))
    y = np.zeros((EMB, N_TOK), np.float64)
    for c in range(HEADS):
        r = res.results[c]
        y += r["y"].astype(np.float64) / r["dn"].astype(np.float64)
    y = (y + np.asarray(b_out, np.float64)[:, None]).astype(np.float32)
    return y.reshape(EMB, H, W)
